# revision 2
# baseline (speedup 1.0000x reference)
"""Batched MoE (dense routing) Trainium2 kernel.

Reference computation (per batch row b):
    alpha = softmax(x @ Wg + bg)                      # (B, E)
    h = relu(x @ W0[e]); h = relu(h @ W1[e]); h = relu(h @ W2[e])
    h3[e] = h @ W3[e]
    y = sum_e alpha[:, e, None] * h3[e]               # (B, 128)

Shapes: B=65536, D=512, E=8, DH=128, DOUT=128.

Strategy: data-parallel shard B across 8 NeuronCores (8192 rows each);
weights replicated.  Host pre-transposes x to [D, B] and re-lays-out the
weight stacks so every device DMA is contiguous.  On device, activations
live in transposed [feature, batch] layout so every matmul streams with
free dim 512 and the per-expert weight is the stationary operand; the
final layer flips back to [batch, out] layout (lhsT = h2 slice) so the
alpha-weighted expert sum is a per-partition-scalar FMA on the vector
engine.  Matmul operands use float32r (1 cycle/row on the PE at free
dim >= 256, ~1.5e-4 relative error vs fp32).
"""

import numpy as np

import concourse.bass as bass
import concourse.tile as tile
from concourse import bacc, mybir
from concourse.bass_utils import run_bass_kernel_spmd

B, D, E, DH, DOUT = 65536, 512, 8, 128, 128
N_CORES = 8
B_LOCAL = B // N_CORES          # 8192
NB = 512                        # batch rows per chunk
CHUNKS = B_LOCAL // NB          # 16
DK = D // 128                   # 4 k-tiles over the input dim
P = 128

F32 = mybir.dt.float32
F32R = mybir.dt.float32r

_CACHE = {}


def _build():
    if "nc" in _CACHE:
        return _CACHE["nc"]

    nc = bacc.Bacc("TRN2", target_bir_lowering=False, debug=False,
                   num_devices=N_CORES)

    xt_ap = nc.dram_tensor("xt", [D, B_LOCAL], F32, kind="ExternalInput").ap()
    w0_ap = nc.dram_tensor("w0", [P, E, DK, DH], F32, kind="ExternalInput").ap()
    w1_ap = nc.dram_tensor("w1", [P, E, DH], F32, kind="ExternalInput").ap()
    w2_ap = nc.dram_tensor("w2", [P, E, DH], F32, kind="ExternalInput").ap()
    w3_ap = nc.dram_tensor("w3", [P, E, DOUT], F32, kind="ExternalInput").ap()
    wg_ap = nc.dram_tensor("wg", [P, DK, E], F32, kind="ExternalInput").ap()
    bg_ap = nc.dram_tensor("bg", [P, NB // P, E], F32, kind="ExternalInput").ap()
    y_ap = nc.dram_tensor("y", [B_LOCAL, DOUT], F32, kind="ExternalOutput").ap()

    BB = NB // P                # 4 batch sub-tiles per chunk

    with tile.TileContext(nc) as tc:
        with (
            tc.tile_pool(name="weights", bufs=1) as wpool,
            tc.tile_pool(name="xt", bufs=3) as xpool,
            tc.tile_pool(name="h", bufs=2) as hpool,
            tc.tile_pool(name="soft", bufs=2) as spool,
            tc.tile_pool(name="acc", bufs=2) as apool,
            tc.tile_pool(name="pg", bufs=2, space="PSUM") as pgpool,
            tc.tile_pool(name="ph", bufs=2, space="PSUM") as phpool,
            tc.tile_pool(name="po", bufs=2, space="PSUM") as popool,
        ):
            w0_sb = wpool.tile([P, E, DK, DH], F32R)
            nc.sync.dma_start(w0_sb[:], w0_ap.bitcast(F32R))
            w1_sb = wpool.tile([P, E, DH], F32R)
            nc.sync.dma_start(w1_sb[:], w1_ap.bitcast(F32R))
            w2_sb = wpool.tile([P, E, DH], F32R)
            nc.sync.dma_start(w2_sb[:], w2_ap.bitcast(F32R))
            w3_sb = wpool.tile([P, E, DOUT], F32R)
            nc.sync.dma_start(w3_sb[:], w3_ap.bitcast(F32R))
            wg_sb = wpool.tile([P, DK, E], F32R)
            nc.sync.dma_start(wg_sb[:], wg_ap.bitcast(F32R))
            bg_sb = wpool.tile([P, BB, E], F32)
            nc.sync.dma_start(bg_sb[:], bg_ap)

            for c in range(CHUNKS):
                # ---- load transposed x chunk: [d_part, dk, b] ----
                xts = xpool.tile([P, DK, NB], F32R)
                nc.sync.dma_start(
                    xts[:],
                    xt_ap[:, c * NB:(c + 1) * NB]
                    .rearrange("(dk p) b -> p dk b", p=P).bitcast(F32R),
                )

                # ---- gate logits: [b_part, bb, e] ----
                pg = pgpool.tile([P, BB, E], F32)
                for bb in range(BB):
                    for dk in range(DK):
                        nc.tensor.matmul(
                            pg[:, bb, :],
                            xts[:, dk, bb * P:(bb + 1) * P],
                            wg_sb[:, dk, :],
                            start=(dk == 0), stop=(dk == DK - 1),
                        )

                # ---- softmax over E (on DVE except exp) ----
                logits = spool.tile([P, BB, E], F32)
                nc.vector.tensor_tensor(logits[:], pg[:], bg_sb[:],
                                        mybir.AluOpType.add)
                expz = spool.tile([P, BB, E], F32)
                nc.scalar.activation(expz[:], logits[:],
                                     mybir.ActivationFunctionType.Exp)
                ssum = spool.tile([P, BB], F32)
                nc.vector.tensor_reduce(ssum[:], expz[:],
                                        axis=mybir.AxisListType.X,
                                        op=mybir.AluOpType.add)
                rs = spool.tile([P, BB], F32)
                nc.vector.reciprocal(rs[:], ssum[:])
                alpha = spool.tile([P, BB, E], F32, tag="alpha")
                nc.vector.tensor_tensor(
                    alpha[:], expz[:],
                    rs[:, :, None].to_broadcast([P, BB, E]),
                    mybir.AluOpType.mult,
                )

                acc = apool.tile([P, BB, DOUT], F32)

                # ---- experts, processed in pairs ----
                for ep in range(E // 2):
                    ph0 = phpool.tile([P, 2, NB], F32, tag="ph")
                    for j in range(2):
                        e = 2 * ep + j
                        for dk in range(DK):
                            nc.tensor.matmul(
                                ph0[:, j, :], w0_sb[:, e, dk, :], xts[:, dk, :],
                                start=(dk == 0), stop=(dk == DK - 1),
                            )
                    h0 = hpool.tile([P, 2, NB], F32R, tag="h0")
                    nc.scalar.activation(h0[:], ph0[:],
                                         mybir.ActivationFunctionType.Relu)

                    ph1 = phpool.tile([P, 2, NB], F32, tag="ph")
                    for j in range(2):
                        e = 2 * ep + j
                        nc.tensor.matmul(ph1[:, j, :], w1_sb[:, e, :],
                                         h0[:, j, :], start=True, stop=True)
                    h1 = hpool.tile([P, 2, NB], F32R, tag="h1")
                    nc.scalar.activation(h1[:], ph1[:],
                                         mybir.ActivationFunctionType.Relu)

                    ph2 = phpool.tile([P, 2, NB], F32, tag="ph")
                    for j in range(2):
                        e = 2 * ep + j
                        nc.tensor.matmul(ph2[:, j, :], w2_sb[:, e, :],
                                         h1[:, j, :], start=True, stop=True)
                    h2 = hpool.tile([P, 2, NB], F32R, tag="h2")
                    nc.vector.tensor_scalar_max(h2[:], ph2[:], 0.0)

                    # final layer back in [b, out] layout + alpha-weighted sum
                    for j in range(2):
                        e = 2 * ep + j
                        po = popool.tile([P, BB, DOUT], F32, tag="po")
                        for bb in range(BB):
                            nc.tensor.matmul(
                                po[:, bb, :],
                                h2[:, j, bb * P:(bb + 1) * P],
                                w3_sb[:, e, :],
                                start=True, stop=True,
                            )
                        for bb in range(BB):
                            if e == 0:
                                nc.vector.tensor_scalar_mul(
                                    acc[:, bb, :], po[:, bb, :],
                                    alpha[:, bb, 0:1])
                            else:
                                nc.vector.scalar_tensor_tensor(
                                    acc[:, bb, :], po[:, bb, :],
                                    alpha[:, bb, e:e + 1], acc[:, bb, :],
                                    mybir.AluOpType.mult, mybir.AluOpType.add,
                                )

                # ---- store chunk output ----
                nc.sync.dma_start(
                    y_ap[c * NB:(c + 1) * NB, :]
                    .rearrange("(bb p) o -> p bb o", p=P),
                    acc[:],
                )

    nc.compile()
    _CACHE["nc"] = nc
    return nc


def _prep_inputs(x, Wg, bg, W0, W1, W2, W3):
    x = np.ascontiguousarray(np.asarray(x, dtype=np.float32))
    Wg = np.asarray(Wg, dtype=np.float32)
    bg = np.asarray(bg, dtype=np.float32)
    W0 = np.asarray(W0, dtype=np.float32)
    W1 = np.asarray(W1, dtype=np.float32)
    W2 = np.asarray(W2, dtype=np.float32)
    W3 = np.asarray(W3, dtype=np.float32)
    assert x.shape == (B, D)

    xt = np.ascontiguousarray(x.T)                                  # [D, B]
    w0h = np.ascontiguousarray(
        W0.reshape(E, DK, P, DH).transpose(2, 0, 1, 3))             # [P,E,DK,DH]
    w1h = np.ascontiguousarray(W1.transpose(1, 0, 2))               # [P,E,DH]
    w2h = np.ascontiguousarray(W2.transpose(1, 0, 2))
    w3h = np.ascontiguousarray(W3.transpose(1, 0, 2))
    wgh = np.ascontiguousarray(Wg.reshape(DK, P, E).transpose(1, 0, 2))
    bgh = np.ascontiguousarray(
        np.broadcast_to(bg, (P, NB // P, E)).astype(np.float32))

    in_maps = []
    for core in range(N_CORES):
        sl = slice(core * B_LOCAL, (core + 1) * B_LOCAL)
        in_maps.append({
            "xt": np.ascontiguousarray(xt[:, sl]),
            "w0": w0h, "w1": w1h, "w2": w2h, "w3": w3h,
            "wg": wgh, "bg": bgh,
        })
    return in_maps


def _run(inputs, trace=False, **kwargs):
    nc = _build()
    in_maps = _prep_inputs(**inputs)
    res = run_bass_kernel_spmd(nc, in_maps, core_ids=list(range(N_CORES)),
                               trace=trace, **kwargs)
    y = np.concatenate([res.results[i]["y"] for i in range(N_CORES)], axis=0)
    return y, res


def kernel(**inputs):
    y, _ = _run(inputs)
    return y


# revision 4
# speedup vs baseline: 1.7334x; 1.7334x over previous
"""Batched MoE (dense routing) Trainium2 kernel.

Reference computation (per batch row b):
    alpha = softmax(x @ Wg + bg)                      # (B, E)
    h = relu(x @ W0[e]); h = relu(h @ W1[e]); h = relu(h @ W2[e])
    h3[e] = h @ W3[e]
    y = sum_e alpha[:, e, None] * h3[e]               # (B, 128)

Shapes: B=65536, D=512, E=8, DH=128, DOUT=128.

Strategy: data-parallel shard B across 8 NeuronCores (8192 rows each);
weights replicated.  Host pre-transposes x to [D, B] and re-lays-out the
weight stacks so every device DMA is contiguous.  On device, activations
live in transposed [feature, batch] layout so every MLP matmul streams
with free dim 512 and the per-expert weight is the stationary operand;
the final layer flips back to [batch, out] layout (lhsT = h2 slice) so
the alpha-weighted expert sum is a per-partition-scalar FMA on the
vector engine.  The gate runs Wg-stationary producing logits in [E, b]
layout with bg+exp fused into one scalar-engine activation, then the
unnormalized exp weights are transposed back to [b, E] with the PE;
normalization (1/sum) is applied once to the accumulated output.

The expert loop is software-pipelined (stage rotation L0 | L1 | L2 |
L3+combine across consecutive experts) so the in-order PE never waits
for a relu eviction.  Matmul operands use float32r (full PE rate at
free dim 512, ~1.5e-4 relative error vs fp32).  All relu evictions run
on the scalar engine; softmax arithmetic and the combine run on the
vector engine.
"""

import numpy as np

import concourse.bass as bass
import concourse.tile as tile
from concourse import bacc, mybir
from concourse.bass_utils import run_bass_kernel_spmd
from concourse.masks import make_identity

B, D, E, DH, DOUT = 65536, 512, 8, 128, 128
N_CORES = 8
B_LOCAL = B // N_CORES          # 8192
NB = 512                        # batch rows per chunk
CHUNKS = B_LOCAL // NB          # 16
DK = D // 128                   # 4 k-tiles over the input dim
P = 128
BB = NB // P                    # 4 batch sub-tiles per chunk

F32 = mybir.dt.float32
F32R = mybir.dt.float32r

_CACHE = {}


def _build():
    if "nc" in _CACHE:
        return _CACHE["nc"]

    nc = bacc.Bacc("TRN2", target_bir_lowering=False, debug=False,
                   num_devices=N_CORES)

    xt_ap = nc.dram_tensor("xt", [D, B_LOCAL], F32, kind="ExternalInput").ap()
    w0_ap = nc.dram_tensor("w0", [P, E, DK, DH], F32, kind="ExternalInput").ap()
    w1_ap = nc.dram_tensor("w1", [P, E, DH], F32, kind="ExternalInput").ap()
    w2_ap = nc.dram_tensor("w2", [P, E, DH], F32, kind="ExternalInput").ap()
    w3_ap = nc.dram_tensor("w3", [P, E, DOUT], F32, kind="ExternalInput").ap()
    wg_ap = nc.dram_tensor("wg", [P, DK, E], F32, kind="ExternalInput").ap()
    bg_ap = nc.dram_tensor("bg", [E, 1], F32, kind="ExternalInput").ap()
    y_ap = nc.dram_tensor("y", [B_LOCAL, DOUT], F32, kind="ExternalOutput").ap()

    with tile.TileContext(nc) as tc:
        with (
            tc.tile_pool(name="weights", bufs=1) as wpool,
            tc.tile_pool(name="xt", bufs=3) as xpool,
            tc.tile_pool(name="h", bufs=3) as hpool,
            tc.tile_pool(name="soft", bufs=2) as spool,
            tc.tile_pool(name="acc", bufs=2) as apool,
            tc.tile_pool(name="ph0", bufs=2, space="PSUM") as ph0pool,
            tc.tile_pool(name="ph12", bufs=2, space="PSUM") as ph12pool,
            tc.tile_pool(name="po", bufs=2, space="PSUM") as popool,
            tc.tile_pool(name="pgate", bufs=1, space="PSUM") as pgpool,
        ):
            w0_sb = wpool.tile([P, E, DK, DH], F32R)
            nc.sync.dma_start(w0_sb[:], w0_ap.bitcast(F32R))
            w1_sb = wpool.tile([P, E, DH], F32R)
            nc.sync.dma_start(w1_sb[:], w1_ap.bitcast(F32R))
            w2_sb = wpool.tile([P, E, DH], F32R)
            nc.sync.dma_start(w2_sb[:], w2_ap.bitcast(F32R))
            w3_sb = wpool.tile([P, E, DOUT], F32R)
            nc.sync.dma_start(w3_sb[:], w3_ap.bitcast(F32R))
            wg_sb = wpool.tile([P, DK, E], F32R)
            nc.sync.dma_start(wg_sb[:], wg_ap.bitcast(F32R))
            bg_sb = wpool.tile([E, 1], F32)
            nc.sync.dma_start(bg_sb[:], bg_ap)
            ident = wpool.tile([E, E], F32)
            make_identity(nc, ident[:])

            for c in range(CHUNKS):
                # ---- load transposed x chunk: [d_part, dk, b] ----
                xts = xpool.tile([P, DK, NB], F32R)
                nc.sync.dma_start(
                    xts[:],
                    xt_ap[:, c * NB:(c + 1) * NB]
                    .rearrange("(dk p) b -> p dk b", p=P).bitcast(F32R),
                )

                # ---- gate: logitsT = Wg.T @ xT  -> [E, b], exp(z+bg) on ACT
                pgt = pgpool.tile([E, NB], F32, tag="pgt")
                for dk in range(DK):
                    nc.tensor.matmul(pgt[:], wg_sb[:, dk, :], xts[:, dk, :],
                                     start=(dk == 0), stop=(dk == DK - 1))
                expzT = spool.tile([E, NB], F32, tag="expzT")
                nc.scalar.activation(expzT[:], pgt[:],
                                     mybir.ActivationFunctionType.Exp,
                                     bias=bg_sb[:, 0:1])

                # transpose unnormalized weights back to [b, (bb, e)]
                expz = spool.tile([P, BB, E], F32, tag="expz")
                for bb in range(BB):
                    ptz = pgpool.tile([P, E], F32, tag="ptz")
                    nc.tensor.transpose(ptz[:], expzT[:, bb * P:(bb + 1) * P],
                                        ident[:])
                    nc.vector.tensor_copy(expz[:, bb, :], ptz[:])
                ssum = spool.tile([P, BB], F32, tag="ssum")
                nc.vector.tensor_reduce(ssum[:], expz[:],
                                        axis=mybir.AxisListType.X,
                                        op=mybir.AluOpType.add)
                rs = spool.tile([P, BB], F32, tag="rs")
                nc.vector.reciprocal(rs[:], ssum[:])

                acc = apool.tile([P, BB, DOUT], F32)

                # ---- software-pipelined expert stages ----
                h0t, h1t, h2t = {}, {}, {}
                for t in range(E + 3):
                    if t < E:                      # L0(e=t)
                        e = t
                        ph0 = ph0pool.tile([P, NB], F32, tag="ph0")
                        for dk in range(DK):
                            nc.tensor.matmul(
                                ph0[:], w0_sb[:, e, dk, :], xts[:, dk, :],
                                start=(dk == 0), stop=(dk == DK - 1))
                        h0t[e] = hpool.tile([P, NB], F32R, tag="h0", name=f"h0_{c}_{e}")
                        nc.scalar.activation(
                            h0t[e][:], ph0[:],
                            mybir.ActivationFunctionType.Relu)
                    if 1 <= t <= E:                # L1(e=t-1)
                        e = t - 1
                        ph1 = ph12pool.tile([P, NB], F32, tag="ph12")
                        nc.tensor.matmul(ph1[:], w1_sb[:, e, :], h0t[e][:],
                                         start=True, stop=True)
                        h1t[e] = hpool.tile([P, NB], F32R, tag="h1", name=f"h1_{c}_{e}")
                        nc.scalar.activation(
                            h1t[e][:], ph1[:],
                            mybir.ActivationFunctionType.Relu)
                        del h0t[e]
                    if 2 <= t <= E + 1:            # L2(e=t-2)
                        e = t - 2
                        ph2 = ph12pool.tile([P, NB], F32, tag="ph12")
                        nc.tensor.matmul(ph2[:], w2_sb[:, e, :], h1t[e][:],
                                         start=True, stop=True)
                        h2t[e] = hpool.tile([P, NB], F32R, tag="h2", name=f"h2_{c}_{e}")
                        nc.scalar.activation(
                            h2t[e][:], ph2[:],
                            mybir.ActivationFunctionType.Relu)
                        del h1t[e]
                    if 3 <= t <= E + 2:            # L3(e=t-3) + combine
                        e = t - 3
                        po = popool.tile([P, BB, DOUT], F32, tag="po")
                        for bb in range(BB):
                            nc.tensor.matmul(
                                po[:, bb, :],
                                h2t[e][:, bb * P:(bb + 1) * P],
                                w3_sb[:, e, :],
                                start=True, stop=True)
                        for bb in range(BB):
                            if e == 0:
                                nc.vector.tensor_scalar_mul(
                                    acc[:, bb, :], po[:, bb, :],
                                    expz[:, bb, 0:1])
                            else:
                                nc.vector.scalar_tensor_tensor(
                                    acc[:, bb, :], po[:, bb, :],
                                    expz[:, bb, e:e + 1], acc[:, bb, :],
                                    mybir.AluOpType.mult,
                                    mybir.AluOpType.add)
                        del h2t[e]

                # normalize by 1/sum(exp) and store
                for bb in range(BB):
                    nc.vector.tensor_scalar_mul(
                        acc[:, bb, :], acc[:, bb, :], rs[:, bb:bb + 1])
                nc.sync.dma_start(
                    y_ap[c * NB:(c + 1) * NB, :]
                    .rearrange("(bb p) o -> p bb o", p=P),
                    acc[:],
                )

    nc.compile()
    _CACHE["nc"] = nc
    return nc


def _prep_inputs(x, Wg, bg, W0, W1, W2, W3):
    x = np.ascontiguousarray(np.asarray(x, dtype=np.float32))
    Wg = np.asarray(Wg, dtype=np.float32)
    bg = np.asarray(bg, dtype=np.float32)
    W0 = np.asarray(W0, dtype=np.float32)
    W1 = np.asarray(W1, dtype=np.float32)
    W2 = np.asarray(W2, dtype=np.float32)
    W3 = np.asarray(W3, dtype=np.float32)
    assert x.shape == (B, D)

    xt = np.ascontiguousarray(x.T)                                  # [D, B]
    w0h = np.ascontiguousarray(
        W0.reshape(E, DK, P, DH).transpose(2, 0, 1, 3))             # [P,E,DK,DH]
    w1h = np.ascontiguousarray(W1.transpose(1, 0, 2))               # [P,E,DH]
    w2h = np.ascontiguousarray(W2.transpose(1, 0, 2))
    w3h = np.ascontiguousarray(W3.transpose(1, 0, 2))
    wgh = np.ascontiguousarray(Wg.reshape(DK, P, E).transpose(1, 0, 2))
    bgh = np.ascontiguousarray(bg.reshape(E, 1))

    in_maps = []
    for core in range(N_CORES):
        sl = slice(core * B_LOCAL, (core + 1) * B_LOCAL)
        in_maps.append({
            "xt": np.ascontiguousarray(xt[:, sl]),
            "w0": w0h, "w1": w1h, "w2": w2h, "w3": w3h,
            "wg": wgh, "bg": bgh,
        })
    return in_maps


def _run(inputs, trace=False, **kwargs):
    nc = _build()
    in_maps = _prep_inputs(**inputs)
    res = run_bass_kernel_spmd(nc, in_maps, core_ids=list(range(N_CORES)),
                               trace=trace, **kwargs)
    y = np.concatenate([res.results[i]["y"] for i in range(N_CORES)], axis=0)
    return y, res


def kernel(**inputs):
    y, _ = _run(inputs)
    return y
